# revision 1
# baseline (speedup 1.0000x reference)
"""Cluster-loss (two-view) Trainium2 kernel.

Math:
    f1n = feat1 / ||feat1||_row ;  f2n = feat2 / ||feat2||_row
    s1 = segsum(f1n, label) ; s2 = segsum(f2n, label) ; counts = bincount(label)
    c1 - c2 = (s1 - s2) / max(counts,1)  -> loss = sum(relu(||c1-c2||^2 - margin))

Key identity: s1 - s2 = segsum(f1n - f2n), so the device only computes ONE
segment sum, of h = f1n - f2n.  The segment sum is a one-hot matmul:
    hsegT[d, c] = sum_t h[t, d] * onehot(label[t])[c]
Per 128-token tile:  lhsT (stationary) = u = (f2*r - f1)  [128tok, 128d] fp16
                     rhs  (moving)     = W = (iota==label)*rs1  [128tok, 1024c] fp16
where r = rs2/rs1, rs_i = 1/||f_i||.  Then u.T @ W = -(h)^T-seg contribution
(rs1 scaling folded into W so normalization costs one pass over the data).
PSUM accumulates all 976 tiles in fp32.  Host: all-reduce 8 cores' partials,
remainder tokens (1M - 8*124928 = 576) in numpy, then counts/hinge/sum.

Perf structure:
 - f tiles DMA-cast fp32->fp16 in flight (SWDGE) so every DVE op runs in a
   16-bit perf mode (2x/4x).
 - All W-build operands fp16 to qualify for the 4x tensor_scalar mode.
 - Two-stage software pipeline: batch b's matmul phase is interleaved with
   batch b+1's sum-of-squares phase so the PE never idles >3.4us (HAM stays
   warm).

Sharding: data-parallel over N; core i gets rows [i*124928, (i+1)*124928).
"""

from contextlib import ExitStack

import numpy as np

import concourse.bass as bass
import concourse.mybir as mybir
import concourse.tile as tile
from concourse import bacc
from concourse.bass_utils import run_bass_kernel_spmd

N_CORES = 8
D = 128
C = 1000
CPAD = 1024          # classes padded to 2 PSUM banks / 2x512 matmuls
P = 128              # tokens per tile (matmul K)
TPB = 16             # tiles per DMA batch (1 MiB per view per batch)
N_BATCHES = 61
N_TILES = N_BATCHES * TPB          # 976
SHARD = N_TILES * P                # 124928 tokens per core
MARGIN = 0.1

F32 = mybir.dt.float32
F16 = mybir.dt.float16
AF = mybir.ActivationFunctionType
OP = mybir.AluOpType


def build_nc(n_batches: int = N_BATCHES):
    n_tiles = n_batches * TPB
    shard = n_tiles * P
    # Bacc (not raw Bass): its compile() spills excess sync waits into
    # EventSemaphore instructions — walrus caps most ISA structs at 1 wait.
    nc = bacc.Bacc("TRN2", target_bir_lowering=False, debug=False)

    f1_d = nc.dram_tensor("f1", [shard, D], F32, kind="ExternalInput")
    f2_d = nc.dram_tensor("f2", [shard, D], F32, kind="ExternalInput")
    lab_d = nc.dram_tensor("lab", [P, n_tiles], F32, kind="ExternalInput")
    iota_d = nc.dram_tensor("iota", [P, CPAD], F16, kind="ExternalInput")
    out_d = nc.dram_tensor("hseg", [D, CPAD], F32, kind="ExternalOutput")

    f1r = f1_d.ap().rearrange("(b t p) d -> b p t d", t=TPB, p=P)
    f2r = f2_d.ap().rearrange("(b t p) d -> b p t d", t=TPB, p=P)

    with tile.TileContext(nc) as tc, ExitStack() as ctx:
        const = ctx.enter_context(tc.tile_pool(name="const", bufs=1))
        fpool = ctx.enter_context(tc.tile_pool(name="fpool", bufs=3))
        sqpool = ctx.enter_context(tc.tile_pool(name="sqpool", bufs=2))
        # dead-store `out` of accum Squares; enough bufs that the accum
        # instruction (1 sync-wait slot) never picks up a WAR wait.
        scratch = ctx.enter_context(tc.tile_pool(name="scratch", bufs=34))
        spool = ctx.enter_context(tc.tile_pool(name="spool", bufs=n_batches))
        upool = ctx.enter_context(tc.tile_pool(name="upool", bufs=6))
        wpool = ctx.enter_context(tc.tile_pool(name="wpool", bufs=6))
        ppool = ctx.enter_context(tc.tile_pool(name="ppool", bufs=1, space="PSUM"))

        iota_sb = const.tile([P, CPAD], F16)
        nc.sync.dma_start(iota_sb[:], iota_d[:])
        lab_sb = const.tile([P, n_tiles], F32)
        nc.sync.dma_start(lab_sb[:], lab_d[:])

        psum = ppool.tile([D, CPAD], F32)

        def emit_load(b):
            f1t = fpool.tile([P, TPB, D], F32, name="f1t")
            nc.sync.dma_start(f1t[:], f1r[b])
            f2t = fpool.tile([P, TPB, D], F32, name="f2t")
            nc.sync.dma_start(f2t[:], f2r[b])
            return f1t, f2t

        def emit_sumsq(st, step):
            """Batched per view: one ACT Square over the batch (fp16 out) and
            one DVE reduce per 128-token group; 4 steps interleave with the
            previous batch's matmul phase."""
            if step == 0:
                sq1 = sqpool.tile([P, TPB * D], F16, name="sq1")
                nc.scalar.activation(sq1[:], st["f1t"][:].rearrange("p t d -> p (t d)"),
                                     AF.Square)
                st["sq1"] = sq1
            elif step == 1:
                nc.vector.tensor_reduce(
                    st["ss1"][:], st["sq1"][:].rearrange("p (t d) -> p t d", d=D),
                    axis=mybir.AxisListType.X, op=OP.add,
                )
            elif step == 2:
                sq2 = sqpool.tile([P, TPB * D], F16, name="sq2")
                nc.scalar.activation(sq2[:], st["f2t"][:].rearrange("p t d -> p (t d)"),
                                     AF.Square)
                st["sq2"] = sq2
            elif step == 3:
                nc.vector.tensor_reduce(
                    st["ss2"][:], st["sq2"][:].rearrange("p (t d) -> p t d", d=D),
                    axis=mybir.AxisListType.X, op=OP.add,
                )

        def emit_stats(st):
            """Batch-level: rs1 and r = rs2/rs1 from ss1/ss2."""
            ss1, ss2 = st["ss1"], st["ss2"]
            sqr1 = spool.tile([P, TPB], F32, name="sqr1")
            nc.scalar.activation(sqr1[:], ss1[:], AF.Sqrt)   # ||f1||
            sqr2 = spool.tile([P, TPB], F32, name="sqr2")
            nc.scalar.activation(sqr2[:], ss2[:], AF.Sqrt)   # ||f2||
            inv1 = spool.tile([P, TPB], F32, name="inv1")
            nc.vector.reciprocal(inv1[:], sqr1[:])           # rs1
            inv2 = spool.tile([P, TPB], F32, name="inv2")
            nc.vector.reciprocal(inv2[:], sqr2[:])           # rs2
            rh = spool.tile([P, TPB], F32, name="rh")
            nc.vector.tensor_tensor(rh[:], sqr1[:], inv2[:], OP.mult)  # rs2/rs1
            st["inv1"], st["rh"] = inv1, rh

        def emit_mm(st, t):
            """Per-tile weights u (GPSIMD), one-hot W (DVE), two matmuls."""
            f1t, f2t = st["f1t"], st["f2t"]
            ti = st["b"] * TPB + t
            # u = f2*r - f1   (= -h/rs1): scale on ACT, subtract on GPSIMD —
            # keeps the DVE free for the one-hot build.
            t2 = upool.tile([P, D], F32, name="t2")
            nc.scalar.activation(
                t2[:], f2t[:, t, :], AF.Copy, bias=0.0,
                scale=st["rh"][:, t : t + 1],
            )
            u = upool.tile([P, D], F16, name="u")
            nc.gpsimd.tensor_tensor(u[:], t2[:], f1t[:, t, :], OP.subtract)
            w = wpool.tile([P, CPAD], F16, name="w")
            nc.vector.tensor_scalar(
                out=w[:], in0=iota_sb[:],
                scalar1=lab_sb[:, ti : ti + 1],
                scalar2=st["inv1"][:, t : t + 1],
                op0=OP.is_equal, op1=OP.mult,
            )
            first = ti == 0
            last = ti == n_tiles - 1
            nc.tensor.matmul(
                psum[:, 0:512], u[:], w[:, 0:512], start=first, stop=last
            )
            nc.tensor.matmul(
                psum[:, 512:CPAD], u[:], w[:, 512:CPAD], start=first, stop=last
            )

        # two-stage software pipeline over batches
        prev = None
        for b in range(n_batches + 1):
            cur = None
            if b < n_batches:
                f1t, f2t = emit_load(b)
                cur = {
                    "b": b, "f1t": f1t, "f2t": f2t,
                    "ss1": spool.tile([P, TPB], F32, name="ss1"),
                    "ss2": spool.tile([P, TPB], F32, name="ss2"),
                }
            for t in range(TPB):
                if prev is not None:
                    emit_mm(prev, t)
                if cur is not None and t % 4 == 1:
                    emit_sumsq(cur, t // 4)
            if cur is not None:
                emit_stats(cur)
            prev = cur

        outsb = const.tile([D, CPAD], F32)
        nc.scalar.copy(outsb[:], psum[:])
        nc.sync.dma_start(out_d[:], outsb[:])

    nc.compile()
    return nc


_NC_CACHE = {}


def _get_nc(n_batches: int = N_BATCHES):
    if n_batches not in _NC_CACHE:
        _NC_CACHE[n_batches] = build_nc(n_batches)
    return _NC_CACHE[n_batches]


def make_in_maps(feat1, feat2, label1, n_batches: int = N_BATCHES):
    shard = n_batches * TPB * P
    iota = np.ascontiguousarray(
        np.broadcast_to(np.arange(CPAD, dtype=np.float16), (P, CPAD))
    )
    in_maps = []
    for c in range(N_CORES):
        lo = c * shard
        lab = (
            label1[lo : lo + shard]
            .astype(np.float32)
            .reshape(n_batches * TPB, P)
            .T.copy()
        )
        in_maps.append(
            {
                "f1": feat1[lo : lo + shard],
                "f2": feat2[lo : lo + shard],
                "lab": lab,
                "iota": iota,
            }
        )
    return in_maps


def finish_host(hsegT_list, feat1, feat2, label1, used: int):
    """Combine per-core partials + host remainder -> scalar loss (float32)."""
    # device psum[d, c] = sum_t (f2*rs2 - f1*rs1)[t, d] * onehot[t, c] = -(s1-s2)^T
    hseg = np.zeros((D, C), dtype=np.float64)
    for h in hsegT_list:
        hseg += h[:, :C].astype(np.float64)
    rem1 = feat1[used:].astype(np.float64)
    rem2 = feat2[used:].astype(np.float64)
    reml = label1[used:]
    if rem1.shape[0]:
        n1 = np.sqrt((rem1 * rem1).sum(1, keepdims=True))
        n2 = np.sqrt((rem2 * rem2).sum(1, keepdims=True))
        hrem = rem1 / n1 - rem2 / n2  # [r, D]
        np.add.at(hseg.T, reml, -hrem)  # device sign convention: -(h)
    counts = np.bincount(label1, minlength=C).astype(np.float64)
    denom = np.maximum(counts, 1.0)
    cdiff = hseg / denom[None, :]
    per_class = (cdiff * cdiff).sum(0)
    hinge = np.maximum(per_class - MARGIN, 0.0)
    hinge = np.where(counts > 0, hinge, 0.0)
    return np.array(hinge.sum(), dtype=np.float32)


def kernel(feat1, feat2, label1, trace: bool = False):
    feat1 = np.ascontiguousarray(np.asarray(feat1, dtype=np.float32))
    feat2 = np.ascontiguousarray(np.asarray(feat2, dtype=np.float32))
    label1 = np.asarray(label1).astype(np.int64)

    in_maps = make_in_maps(feat1, feat2, label1)
    nc = _get_nc()
    res = run_bass_kernel_spmd(
        nc, in_maps, core_ids=list(range(N_CORES)), trace=trace
    )
    hsegs = [res.results[i]["hseg"] for i in range(N_CORES)]
    out = finish_host(hsegs, feat1, feat2, label1, used=N_CORES * SHARD)
    if trace:
        return out, res
    return out



# revision 10
# speedup vs baseline: 2.3313x; 2.3313x over previous
"""Cluster-loss (two-view) Trainium2 kernel — sorted/windowed segment sum.

Math:
    f1n = feat1 / ||feat1||_row ;  f2n = feat2 / ||feat2||_row
    s1 = segsum(f1n, label) ; s2 = segsum(f2n, label) ; counts = bincount(label)
    loss = sum(relu(||(s1-s2)/max(counts,1)||^2 - margin))

Strategy (vs the one-hot-over-1024-classes baseline):
  Host sorts tokens by label.  A 128-token tile of sorted data spans ~1.2 of
  the 1000 classes, so the segment-sum matmul only needs a W=32-class window
  per tile instead of 1024 columns:
      psum[d, base_t : base_t+32] += f_v[tile].T @ w_v[tile]      (N=32)
  with w_v[t, j] = (j == label_rel[t]) * rs_v[t],  rs_v = 1/||f_v||.
  Window bases are data-INDEPENDENT (base(t) = floor(t*125/976), origin
  125*core - 8) so one SPMD program serves all cores; sorted-uniform labels
  stay well inside the +-8..+21 margin, and any token that misses its window
  is excluded on device (rel = -1 never matches) and added exactly on host.

  Per-token normalization runs on device: ACT Square -> two DVE half-adds ->
  DVE reduce -> rs = ss^-0.5 via DVE tensor_scalar pow.  The one-hot builds
  are batched: one broadcast TT subtract (iota - rel) + one scalar_tensor_
  tensor (is_equal 0) * rs per 16-tile batch per view.

  View1 and view2 accumulate into separate PSUM halves of one bank; the
  device output is psumA - psumB = (s1 - s2)^T over the core's 256-class
  window.  Counts, the 576 leftover tokens, and the hinge are host-side.

Sharding: data-parallel over sorted N; core i gets rows [i*124928, (i+1)*124928).
All DMA is fp16 host-prepacked partition-major (4 KiB contiguous lines).
"""

from contextlib import ExitStack

import numpy as np

import concourse.bass as bass
import concourse.mybir as mybir
import concourse.tile as tile
from concourse import bacc
from concourse.bass_utils import run_bass_kernel_spmd

N_CORES = 8
D = 128
C = 1000
P = 128              # tokens per tile (matmul K)
TPB = 16             # tiles per batch
NB = 61              # batches
NT = NB * TPB        # 976 tiles
SHARD = NT * P       # 124928 tokens per core
USED = N_CORES * SHARD
W = 16               # class window per tile
CPAD = 256           # per-core class window (PSUM half-bank)
CSTEP = 125          # per-core class-origin stride
MARGIN_SLACK = 8     # window starts this many classes below prediction
MARGIN = 0.1

F32 = mybir.dt.float32
F16 = mybir.dt.float16
AF = mybir.ActivationFunctionType
OP = mybir.AluOpType

# base_rel[t]: window start for tile t, relative to the core's class origin.
BASE_REL = [(t * CSTEP) // NT for t in range(NT)]


def build_nc():
    nc = bacc.Bacc("TRN2", target_bir_lowering=False, debug=False)

    f1_d = nc.dram_tensor("f1", [P, NT * D], F16, kind="ExternalInput")
    f2_d = nc.dram_tensor("f2", [P, NT * D], F16, kind="ExternalInput")
    rel_d = nc.dram_tensor("rel", [P, NT], F16, kind="ExternalInput")
    iota_d = nc.dram_tensor("iota", [P, W], F16, kind="ExternalInput")
    out_d = nc.dram_tensor("hseg", [D, CPAD], F32, kind="ExternalOutput")

    with tile.TileContext(nc) as tc, ExitStack() as ctx:
        const = ctx.enter_context(tc.tile_pool(name="const", bufs=1))
        fpool = ctx.enter_context(tc.tile_pool(name="fpool", bufs=3))
        sqpool = ctx.enter_context(tc.tile_pool(name="sqpool", bufs=2))
        hpool = ctx.enter_context(tc.tile_pool(name="hpool", bufs=2))
        spool = ctx.enter_context(tc.tile_pool(name="spool", bufs=3))
        wpool = ctx.enter_context(tc.tile_pool(name="wpool", bufs=3))
        ppool = ctx.enter_context(tc.tile_pool(name="ppool", bufs=1, space="PSUM"))

        iota_sb = const.tile([P, W], F16)
        nc.sync.dma_start(iota_sb[:], iota_d[:])
        rel_sb = const.tile([P, NT], F16)
        nc.sync.dma_start(rel_sb[:], rel_d[:])
        zeros = const.tile([P, CPAD], F16)
        nc.gpsimd.memset(zeros[:], 0.0)

        # Both views accumulate into one PSUM region (view2 with negated rs).
        psum = ppool.tile([D, CPAD], F32)
        # Zero-matmul marks the region written so windowed accumulating
        # matmuls (start=False) hit defined values everywhere.
        nc.tensor.matmul(
            psum[:], zeros[:, 0:P], zeros[:], start=True, stop=False
        )

        def emit_sumsq(ft, ss_slice, view):
            """ss = sum_d f^2 per token: ACT square then DVE half-add tree."""
            sq = sqpool.tile([P, TPB, D], F16, name=f"sq{view}")
            nc.scalar.activation(sq[:], ft[:], AF.Square)
            h1 = hpool.tile([P, TPB, D // 2], F16, name=f"h1{view}")
            nc.vector.tensor_tensor(h1[:], sq[:, :, 0:64], sq[:, :, 64:128], OP.add)
            h2 = hpool.tile([P, TPB, D // 4], F16, name=f"h2{view}")
            nc.vector.tensor_tensor(h2[:], h1[:, :, 0:32], h1[:, :, 32:64], OP.add)
            nc.vector.tensor_reduce(
                ss_slice, h2[:], axis=mybir.AxisListType.X, op=OP.add
            )

        for b in range(NB):
            f1t = fpool.tile([P, TPB, D], F16, name="f1t")
            nc.sync.dma_start(
                f1t[:], f1_d[:, b * TPB * D : (b + 1) * TPB * D].rearrange(
                    "p (t d) -> p t d", d=D)
            )
            f2t = fpool.tile([P, TPB, D], F16, name="f2t")
            nc.sync.dma_start(
                f2t[:], f2_d[:, b * TPB * D : (b + 1) * TPB * D].rearrange(
                    "p (t d) -> p t d", d=D)
            )

            # Both views' norms through one Sqrt + one reciprocal per batch.
            ss = spool.tile([P, 2, TPB], F32, name="ss")
            emit_sumsq(f1t, ss[:, 0, :], 1)
            emit_sumsq(f2t, ss[:, 1, :], 2)
            sqr = spool.tile([P, 2, TPB], F32, name="sqr")
            nc.scalar.activation(sqr[:], ss[:], AF.Sqrt)
            rs = spool.tile([P, 2, TPB], F32, name="rs")
            nc.vector.reciprocal(rs[:], sqr[:])
            rs1 = rs[:, 0, :]
            rs2n = spool.tile([P, TPB], F32, name="rs2n")
            nc.vector.tensor_scalar(
                out=rs2n[:], in0=rs[:, 1, :], scalar1=-1.0, scalar2=None,
                op0=OP.mult,
            )

            # Batched one-hot * rs for all 16 tiles of both views.
            diff = wpool.tile([P, TPB, W], F16, name="diff")
            nc.vector.tensor_tensor(
                diff[:],
                iota_sb[:].unsqueeze(1).broadcast_to([P, TPB, W]),
                rel_sb[:, b * TPB : (b + 1) * TPB].unsqueeze(2).broadcast_to(
                    [P, TPB, W]),
                OP.subtract,
            )
            w1 = wpool.tile([P, TPB, W], F16, name="w1")
            nc.vector.scalar_tensor_tensor(
                w1[:], diff[:], 0.0, rs1.unsqueeze(2).broadcast_to([P, TPB, W]),
                op0=OP.is_equal, op1=OP.mult,
            )
            w2 = wpool.tile([P, TPB, W], F16, name="w2")
            nc.vector.scalar_tensor_tensor(
                w2[:], diff[:], 0.0, rs2n[:].unsqueeze(2).broadcast_to([P, TPB, W]),
                op0=OP.is_equal, op1=OP.mult,
            )

            last = b == NB - 1
            for t in range(TPB):
                b0 = BASE_REL[b * TPB + t]
                # stop only on the final matmul: all matmuls share one PSUM
                # zero region, so an earlier stop would end the group.
                nc.tensor.matmul(
                    psum[:, b0 : b0 + W], f1t[:, t, :], w1[:, t, :],
                    start=False, stop=False,
                )
                nc.tensor.matmul(
                    psum[:, b0 : b0 + W], f2t[:, t, :], w2[:, t, :],
                    start=False, stop=last and t == TPB - 1,
                )

        outsb = const.tile([D, CPAD], F32)
        nc.vector.tensor_copy(outsb[:], psum[:])
        nc.sync.dma_start(out_d[:], outsb[:])

    nc.compile()
    return nc


_NC_CACHE = {}


def _get_nc():
    if "nc" not in _NC_CACHE:
        _NC_CACHE["nc"] = build_nc()
    return _NC_CACHE["nc"]


def prepare_inputs(feat1, feat2, label1):
    """Sort by label, pack per-core fp16 partition-major inputs, and collect
    host-handled token indices (sorted-order tail + window misses)."""
    order = np.argsort(label1, kind="stable").astype(np.int64)
    labs = label1[order]

    iota = np.ascontiguousarray(
        np.broadcast_to(np.arange(W, dtype=np.float16), (P, W))
    )
    in_maps = []
    host_tokens = [order[USED:]]  # sorted tail not sent to any core
    for c in range(N_CORES):
        sl = slice(c * SHARD, (c + 1) * SHARD)
        idx = order[sl].reshape(NT, P).T          # [P, NT] token ids
        core_labs = labs[sl].reshape(NT, P).T     # [P, NT]
        origin = c * CSTEP - MARGIN_SLACK
        rel = core_labs - origin - np.asarray(BASE_REL, dtype=np.int64)[None, :]
        miss = (rel < 0) | (rel >= W)
        if miss.any():
            host_tokens.append(idx[miss])
            rel = np.where(miss, -1, rel)
        f1p = feat1[idx].astype(np.float16).reshape(P, NT * D)
        f2p = feat2[idx].astype(np.float16).reshape(P, NT * D)
        in_maps.append(
            {
                "f1": f1p,
                "f2": f2p,
                "rel": rel.astype(np.float16),
                "iota": iota,
            }
        )
    return in_maps, np.concatenate(host_tokens)


def finish_host(hseg_list, feat1, feat2, label1, host_tokens):
    """Per-core windowed partials + host-handled tokens -> scalar loss."""
    hseg = np.zeros((D, C), dtype=np.float64)
    for c, part in enumerate(hseg_list):
        origin = c * CSTEP - MARGIN_SLACK
        j0 = max(0, -origin)
        j1 = min(CPAD, C - origin)
        hseg[:, origin + j0 : origin + j1] += part[:, j0:j1].astype(np.float64)
    if host_tokens.size:
        r1 = feat1[host_tokens].astype(np.float64)
        r2 = feat2[host_tokens].astype(np.float64)
        n1 = np.sqrt((r1 * r1).sum(1, keepdims=True))
        n2 = np.sqrt((r2 * r2).sum(1, keepdims=True))
        hrem = r1 / n1 - r2 / n2
        np.add.at(hseg.T, label1[host_tokens], hrem)
    counts = np.bincount(label1, minlength=C).astype(np.float64)
    denom = np.maximum(counts, 1.0)
    cdiff = hseg / denom[None, :]
    per_class = (cdiff * cdiff).sum(0)
    hinge = np.maximum(per_class - MARGIN, 0.0)
    hinge = np.where(counts > 0, hinge, 0.0)
    return np.array(hinge.sum(), dtype=np.float32)


def kernel(feat1, feat2, label1, trace: bool = False):
    feat1 = np.ascontiguousarray(np.asarray(feat1, dtype=np.float32))
    feat2 = np.ascontiguousarray(np.asarray(feat2, dtype=np.float32))
    label1 = np.asarray(label1).astype(np.int64)

    in_maps, host_tokens = prepare_inputs(feat1, feat2, label1)
    nc = _get_nc()
    res = run_bass_kernel_spmd(
        nc, in_maps, core_ids=list(range(N_CORES)), trace=trace
    )
    hsegs = [res.results[i]["hseg"] for i in range(N_CORES)]
    out = finish_host(hsegs, feat1, feat2, label1, host_tokens)
    if trace:
        return out, res
    return out
